# revision 22
# baseline (speedup 1.0000x reference)
"""ACDC2D fused kernel for 8 Trainium2 NeuronCores.

The reference module is: grouped 3x3 conv (64ch, 8 groups) -> *A ->
FFT(channel) -> *D -> IFFT(channel) -> +bias -> channel permutation ->
/sqrt(64), producing a complex64 output.

Because ifft(D * fft(z)) along a length-64 axis is multiplication by the
circulant matrix C = circ(ifft(D)), the whole module collapses to a single
dense 3x3 convolution with 64 input channels and 128 output channels
(real part || imag part), whose weights are precomputed on the host from
(conv_w, A, D, perm); the permutation, A, D and 1/8 scale all fold into
the weights, and the (real) bias is added during PSUM evacuation.

Device kernel (per core, data-parallel over batch: 2 samples/core):
  - The host pre-pads x to 194x194 (one zero guard row/col each side) in
    bf16, so every conv tap is a pure flat offset into SBUF and every DMA
    is one contiguous multi-KB run per partition.
  - Sample 0 lives on SBUF partitions 0-63, sample 1 on partitions
    64-127. Each conv tap is a K=64 matmul on its own PE row-group
    (tile_position (0,0) / (64,0)); interleaving the two samples' taps
    makes consecutive matmuls run CONCURRENTLY on the two halves of the
    128x128 array, so 18 matmuls take ~9 passes of the free dim.
    Input is read once, no duplicated copies, no shifted SBUF copies.
  - Each sample's 9 taps accumulate into that sample's own PSUM bank per
    512-col chunk; ScalarE evacuates sample 0 and VectorE sample 1
    (PSUM -> SBUF bf16, + per-channel bias), DMA writes padded rows out.
Host post-processing strips the pad and assembles complex64.
"""

import numpy as np
import ml_dtypes

import concourse.bass as bass
import concourse.bacc as bacc
import concourse.tile as tile
from concourse import mybir
from concourse.bass_utils import run_bass_kernel_spmd

# ---- problem geometry (hardcoded, matches setup_inputs) ----
B, CIN, H, W = 16, 64, 192, 192
COUT = 128               # 64 real + 64 imag output channels
NCORES = 8
BS = B // NCORES         # samples per core (2: one per partition half)
Wp = W + 1               # padded row width (SHARED guard: right guard of row h = left guard of row h+1)
Hp = H + 2               # padded height (guard row top/bottom)
TH = 32                  # output rows per strip
NR = TH + 4              # row slots per SBUF region (1 pad front/back)
NSTRIP = H // TH
SPAN0, SPAN1 = 2 * Wp, (TH + 2) * Wp   # matmul span within a strip buffer
CHUNK = 512              # psum bank / matmul free-dim limit (fp32 psum)
ROWS_A = 8               # rows in the first input-DMA piece (startup pipelining)

BF16 = mybir.dt.bfloat16
F32 = mybir.dt.float32

TAPS = [(dh, dw) for dh in (-1, 0, 1) for dw in (-1, 0, 1)]

_nc_cache = {}


def _fused_weights(conv_w, A, D, bias, perm):
    """Fold conv + A + FFT*D*IFFT + perm + 1/8 into dense conv weights.

    Returns (wts[k=128, tap=9, o=128] f32 — identical halves, and
    bvec[128,1] f32).
    """
    A = np.asarray(A, np.float64).reshape(CIN)
    D = np.asarray(D, np.float64).reshape(CIN)
    bias = np.asarray(bias, np.float64).reshape(CIN)
    perm = np.asarray(perm).reshape(CIN)
    conv_w = np.asarray(conv_w, np.float64)          # (64, 8, 3, 3)

    c = np.fft.ifft(D)
    idx = (np.arange(CIN)[:, None] - np.arange(CIN)[None, :]) % CIN
    M = (c[idx] * A[None, :])[perm, :] / 8.0          # complex (64,64)
    bias_p = (bias[perm] / 8.0).astype(np.float32)

    W_d = np.zeros((CIN, CIN, 3, 3))
    for co in range(CIN):
        g = co // 8
        W_d[co, g * 8:(g + 1) * 8] = conv_w[co]
    U = np.einsum("oc,cikl->oikl", M, W_d)            # complex (64,64,3,3)
    W2 = np.concatenate([np.real(U), np.imag(U)], axis=0).astype(np.float32)

    wts = np.zeros((128, 9, 128), np.float32)         # [k, tap, o]
    for t, (dh, dw) in enumerate(TAPS):
        lt = W2[:, :, dh + 1, dw + 1].T               # [i, o]
        wts[:64, t, :] = lt
        wts[64:, t, :] = lt
    bvec = np.concatenate([bias_p, np.zeros(64, np.float32)]).reshape(128, 1)
    return wts, bvec


def _build_program():
    nc = bacc.Bacc("TRN2", target_bir_lowering=False, debug=False)

    x_d = nc.declare_dram_parameter("xb", [BS, CIN, Hp, Wp], BF16, False)
    w_d = nc.declare_dram_parameter("wts", [128, 9, 128], BF16, False)
    b_d = nc.declare_dram_parameter("bvec", [128, 1], F32, False)
    o_d = nc.declare_dram_parameter("out", [BS, COUT, H, Wp], BF16, True)

    with tile.TileContext(nc) as tc:
        with (
            tc.tile_pool(name="const", bufs=1) as constp,
            tc.tile_pool(name="rin", bufs=3) as rinp,
            tc.tile_pool(name="outp", bufs=4) as outp,
            tc.tile_pool(name="psum", bufs=4, space=bass.MemorySpace.PSUM) as psp,
        ):
            # weights first on the fast sync queue (first matmul needs them);
            # bias on gpsimd (only the first evacuation needs it)
            w_sb = constp.tile([128, 9, 128], BF16)
            nc.sync.dma_start(w_sb[:], w_d[:])
            b_sb = constp.tile([128, 1], F32)
            nc.gpsimd.dma_start(b_sb[:], b_d[:])

            # PE warmup: dep-free matmuls on scratch garbage release the
            # HAM clock gate (1.2 -> 2.4 GHz) while the first input DMAs
            # are still in flight, so real matmuls start warm.
            warm_w = constp.tile([128, 128], BF16)
            nc.gpsimd.memset(warm_w[:], 0.0)
            warm_x = constp.tile([128, CHUNK], BF16)
            nc.gpsimd.memset(warm_x[:], 0.0)
            for wi in range(8):
                warm_ps = psp.tile([128, CHUNK], F32, tag=f"ps{wi % 2}",
                                   name="warmps")
                nc.tensor.matmul(warm_ps[:], warm_w[:], warm_x[:],
                                 start=True, stop=True)

            for si in range(NSTRIP):
                r0 = si * TH

                R = rinp.tile([128, NR, Wp], BF16, tag="rin")
                # slots 1..TH+2 <- padded rows r0..r0+TH+1, one sample per
                # partition half — (s c) merges contiguously into the
                # partition dim, so one DMA covers both samples. Two row
                # pieces so the first chunks' matmuls start before the
                # whole strip has landed.
                nc.sync.dma_start(
                    R[:, 1:1 + ROWS_A, :].rearrange("p r w -> p (r w)"),
                    x_d[:, :, r0:r0 + ROWS_A, :]
                    .rearrange("s c r w -> (s c) (r w)"),
                )
                nc.sync.dma_start(
                    R[:, 1 + ROWS_A:TH + 3, :].rearrange("p r w -> p (r w)"),
                    x_d[:, :, r0 + ROWS_A:r0 + TH + 2, :]
                    .rearrange("s c r w -> (s c) (r w)"),
                )
                # the one guard cell past the last loaded row (read by the
                # (+1,+1) tap at wo=191 of the last output row) must be zero
                nc.gpsimd.memset(R[:, TH + 3:TH + 4, 0:1], 0.0)
                Rf = R[:].rearrange("p r c -> p (r c)")

                OT = [outp.tile([128, TH, Wp], BF16, tag=f"out{h}", name=f"ot{h}")
                      for h in range(BS)]
                OTf = [t[:].rearrange("p r c -> p (r c)") for t in OT]

                pos = SPAN0
                while pos < SPAN1:
                    rem = SPAN1 - pos
                    # avoid a tiny ragged tail chunk (instruction-floor
                    # bound): split the last 512+32 into two 272s
                    n = 272 if rem == CHUNK + 32 else min(CHUNK, rem)
                    ps = [psp.tile([128, CHUNK], F32, tag=f"ps{h}", name=f"ps{h}")
                          for h in range(BS)]
                    for t, (dh, dw) in enumerate(TAPS):
                        off = dh * Wp + dw
                        for h in range(BS):
                            p0 = 64 * h
                            nc.tensor.matmul(
                                ps[h][:, 0:n],
                                w_sb[p0:p0 + 64, t, :],
                                Rf[p0:p0 + 64, pos + off:pos + off + n],
                                start=(t == 0),
                                stop=(t == 8),
                            )
                    lo = pos - SPAN0
                    nc.scalar.activation(
                        OTf[0][:, lo:lo + n], ps[0][:, 0:n],
                        mybir.ActivationFunctionType.Identity,
                        bias=b_sb[:, 0:1],
                    )
                    nc.vector.tensor_scalar_add(
                        OTf[1][:, lo:lo + n], ps[1][:, 0:n], b_sb[:, 0:1]
                    )
                    pos += n

                # output in row-pieces; range-precise dep tracking releases
                # each piece as soon as its chunks are evacuated. Finer
                # pieces on the last strip shrink the post-compute tail.
                cuts = [0, 16, 28, TH] if si == NSTRIP - 1 else [0, 16, TH]
                for h in range(BS):
                    for a, b in zip(cuts, cuts[1:]):
                        # the final pieces' DIRECT2D dispatch (~0.6us each)
                        # sits on the tail critical path: issue sample 1's
                        # on the gpsimd queue so the two dispatch in parallel
                        eng = nc.gpsimd if (h == 1 and si == NSTRIP - 1) else nc.sync
                        eng.dma_start(
                            o_d[h, :, r0 + a:r0 + b, :]
                            .rearrange("c r w -> c (r w)"),
                            OT[h][:, a:b, :].rearrange("p r w -> p (r w)"),
                        )

    nc.compile()
    return nc


def kernel(x, conv_w, A, D, bias, perm):
    x = np.asarray(x)
    wts, bvec = _fused_weights(conv_w, A, D, bias, perm)
    wts_bf = wts.astype(ml_dtypes.bfloat16)

    # host pre-pad: zero guard row/col on each side, bf16
    xpad = np.zeros((B, CIN, Hp, Wp), ml_dtypes.bfloat16)
    xpad[:, :, 1:H + 1, 1:W + 1] = x.astype(ml_dtypes.bfloat16)

    if "nc" not in _nc_cache:
        _nc_cache["nc"] = _build_program()
    nc = _nc_cache["nc"]

    in_maps = [
        {"xb": xpad[c * BS:(c + 1) * BS], "wts": wts_bf, "bvec": bvec}
        for c in range(NCORES)
    ]
    res = run_bass_kernel_spmd(nc, in_maps, core_ids=list(range(NCORES)))
    _nc_cache["last_result"] = res

    out = np.empty((B, 64, H, W), np.complex64)
    for c in range(NCORES):
        o = np.asarray(res.results[c]["out"])        # (BS,128,H,Wp) bf16
        o = o[:, :, :, 1:W + 1].astype(np.float32)
        out[c * BS:(c + 1) * BS].real = o[:, :64]
        out[c * BS:(c + 1) * BS].imag = o[:, 64:]
    return out


# revision 23
# speedup vs baseline: 1.0230x; 1.0230x over previous
"""ACDC2D fused kernel for 8 Trainium2 NeuronCores.

The reference module is: grouped 3x3 conv (64ch, 8 groups) -> *A ->
FFT(channel) -> *D -> IFFT(channel) -> +bias -> channel permutation ->
/sqrt(64), producing a complex64 output.

Because ifft(D * fft(z)) along a length-64 axis is multiplication by the
circulant matrix C = circ(ifft(D)), the whole module collapses to a single
dense 3x3 convolution with 64 input channels and 128 output channels
(real part || imag part), whose weights are precomputed on the host from
(conv_w, A, D, perm); the permutation, A, D and 1/8 scale all fold into
the weights, and the (real) bias is added during PSUM evacuation.

Device kernel (per core, data-parallel over batch: 2 samples/core):
  - The host pre-pads x to 194x194 (one zero guard row/col each side) in
    bf16, so every conv tap is a pure flat offset into SBUF and every DMA
    is one contiguous multi-KB run per partition.
  - Sample 0 lives on SBUF partitions 0-63, sample 1 on partitions
    64-127. Each conv tap is a K=64 matmul on its own PE row-group
    (tile_position (0,0) / (64,0)); interleaving the two samples' taps
    makes consecutive matmuls run CONCURRENTLY on the two halves of the
    128x128 array, so 18 matmuls take ~9 passes of the free dim.
    Input is read once, no duplicated copies, no shifted SBUF copies.
  - Each sample's 9 taps accumulate into that sample's own PSUM bank per
    512-col chunk; ScalarE evacuates sample 0 and VectorE sample 1
    (PSUM -> SBUF bf16, + per-channel bias), DMA writes padded rows out.
Host post-processing strips the pad and assembles complex64.
"""

import numpy as np
import ml_dtypes

import concourse.bass as bass
import concourse.bacc as bacc
import concourse.tile as tile
from concourse import mybir
from concourse.bass_utils import run_bass_kernel_spmd

# ---- problem geometry (hardcoded, matches setup_inputs) ----
B, CIN, H, W = 16, 64, 192, 192
COUT = 128               # 64 real + 64 imag output channels
NCORES = 8
BS = B // NCORES         # samples per core (2: one per partition half)
Wp = W + 1               # padded row width (SHARED guard: right guard of row h = left guard of row h+1)
Hp = H + 2               # padded height (guard row top/bottom)
TH = 32                  # output rows per strip
NR = TH + 4              # row slots per SBUF region (1 pad front/back)
NSTRIP = H // TH
SPAN0, SPAN1 = 2 * Wp, (TH + 2) * Wp   # matmul span within a strip buffer
CHUNK = 512              # psum bank / matmul free-dim limit (fp32 psum)
ROWS_A = 8               # rows in the first input-DMA piece (startup pipelining)

BF16 = mybir.dt.bfloat16
F32 = mybir.dt.float32

TAPS = [(dh, dw) for dh in (-1, 0, 1) for dw in (-1, 0, 1)]

_nc_cache = {}


def _fused_weights(conv_w, A, D, bias, perm):
    """Fold conv + A + FFT*D*IFFT + perm + 1/8 into dense conv weights.

    Returns (wts[k=128, tap=9, o=128] f32 — identical halves, and
    bvec[128,1] f32).
    """
    A = np.asarray(A, np.float64).reshape(CIN)
    D = np.asarray(D, np.float64).reshape(CIN)
    bias = np.asarray(bias, np.float64).reshape(CIN)
    perm = np.asarray(perm).reshape(CIN)
    conv_w = np.asarray(conv_w, np.float64)          # (64, 8, 3, 3)

    c = np.fft.ifft(D)
    idx = (np.arange(CIN)[:, None] - np.arange(CIN)[None, :]) % CIN
    M = (c[idx] * A[None, :])[perm, :] / 8.0          # complex (64,64)
    bias_p = (bias[perm] / 8.0).astype(np.float32)

    W_d = np.zeros((CIN, CIN, 3, 3))
    for co in range(CIN):
        g = co // 8
        W_d[co, g * 8:(g + 1) * 8] = conv_w[co]
    U = np.einsum("oc,cikl->oikl", M, W_d)            # complex (64,64,3,3)
    W2 = np.concatenate([np.real(U), np.imag(U)], axis=0).astype(np.float32)

    wts = np.zeros((128, 9, 128), np.float32)         # [k, tap, o]
    for t, (dh, dw) in enumerate(TAPS):
        lt = W2[:, :, dh + 1, dw + 1].T               # [i, o]
        wts[:64, t, :] = lt
        wts[64:, t, :] = lt
    bvec = np.concatenate([bias_p, np.zeros(64, np.float32)]).reshape(128, 1)
    return wts, bvec


def _build_program():
    nc = bacc.Bacc("TRN2", target_bir_lowering=False, debug=False)

    x_d = nc.declare_dram_parameter("xb", [BS, CIN, Hp, Wp], BF16, False)
    w_d = nc.declare_dram_parameter("wts", [128, 9, 128], BF16, False)
    b_d = nc.declare_dram_parameter("bvec", [128, 1], F32, False)
    o_d = nc.declare_dram_parameter("out", [BS, COUT, H, Wp], BF16, True)

    with tile.TileContext(nc) as tc:
        with (
            tc.tile_pool(name="const", bufs=1) as constp,
            tc.tile_pool(name="rin", bufs=3) as rinp,
            tc.tile_pool(name="outp", bufs=4) as outp,
            tc.tile_pool(name="psum", bufs=4, space=bass.MemorySpace.PSUM) as psp,
        ):
            # weights first on the fast sync queue (first matmul needs them);
            # bias on gpsimd (only the first evacuation needs it)
            w_sb = constp.tile([128, 9, 128], BF16)
            nc.sync.dma_start(w_sb[:], w_d[:])
            b_sb = constp.tile([128, 1], F32)
            nc.gpsimd.dma_start(b_sb[:], b_d[:])

            # PE warmup: dep-free matmuls on scratch garbage release the
            # HAM clock gate (1.2 -> 2.4 GHz) while the first input DMAs
            # are still in flight, so real matmuls start warm.
            warm_w = constp.tile([128, 128], BF16)
            nc.gpsimd.memset(warm_w[:], 0.0)
            warm_x = constp.tile([128, CHUNK], BF16)
            nc.gpsimd.memset(warm_x[:], 0.0)
            for wi in range(8):
                warm_ps = psp.tile([128, CHUNK], F32, tag=f"ps{wi % 2}",
                                   name="warmps")
                nc.tensor.matmul(warm_ps[:], warm_w[:], warm_x[:],
                                 start=True, stop=True)

            for si in range(NSTRIP):
                r0 = si * TH

                R = rinp.tile([128, NR, Wp], BF16, tag="rin")
                # slots 1..TH+2 <- padded rows r0..r0+TH+1, one sample per
                # partition half — (s c) merges contiguously into the
                # partition dim, so one DMA covers both samples. Two row
                # pieces so the first chunks' matmuls start before the
                # whole strip has landed.
                nc.sync.dma_start(
                    R[:, 1:1 + ROWS_A, :].rearrange("p r w -> p (r w)"),
                    x_d[:, :, r0:r0 + ROWS_A, :]
                    .rearrange("s c r w -> (s c) (r w)"),
                )
                nc.sync.dma_start(
                    R[:, 1 + ROWS_A:TH + 3, :].rearrange("p r w -> p (r w)"),
                    x_d[:, :, r0 + ROWS_A:r0 + TH + 2, :]
                    .rearrange("s c r w -> (s c) (r w)"),
                )
                # the one guard cell past the last loaded row (read by the
                # (+1,+1) tap at wo=191 of the last output row) must be zero
                nc.gpsimd.memset(R[:, TH + 3:TH + 4, 0:1], 0.0)
                Rf = R[:].rearrange("p r c -> p (r c)")

                OT = [outp.tile([128, TH, Wp], BF16, tag=f"out{h}", name=f"ot{h}")
                      for h in range(BS)]
                OTf = [t[:].rearrange("p r c -> p (r c)") for t in OT]

                pos = SPAN0
                while pos < SPAN1:
                    rem = SPAN1 - pos
                    # avoid a tiny ragged tail chunk (instruction-floor
                    # bound): split the last 512+32 into two 272s
                    n = 272 if rem == CHUNK + 32 else min(CHUNK, rem)
                    ps = [psp.tile([128, CHUNK], F32, tag=f"ps{h}", name=f"ps{h}")
                          for h in range(BS)]
                    for t, (dh, dw) in enumerate(TAPS):
                        off = dh * Wp + dw
                        for h in range(BS):
                            p0 = 64 * h
                            nc.tensor.matmul(
                                ps[h][:, 0:n],
                                w_sb[p0:p0 + 64, t, :],
                                Rf[p0:p0 + 64, pos + off:pos + off + n],
                                start=(t == 0),
                                stop=(t == 8),
                            )
                    lo = pos - SPAN0
                    nc.scalar.activation(
                        OTf[0][:, lo:lo + n], ps[0][:, 0:n],
                        mybir.ActivationFunctionType.Identity,
                        bias=b_sb[:, 0:1],
                    )
                    nc.vector.tensor_scalar_add(
                        OTf[1][:, lo:lo + n], ps[1][:, 0:n], b_sb[:, 0:1]
                    )
                    pos += n

                # output in row-pieces; range-precise dep tracking releases
                # each piece as soon as its chunks are evacuated. Finer
                # pieces on the last strip shrink the post-compute tail.
                cuts = [0, 16, 28, TH] if si == NSTRIP - 1 else [0, 16, TH]
                for h in range(BS):
                    for a, b in zip(cuts, cuts[1:]):
                        nc.sync.dma_start(
                            o_d[h, :, r0 + a:r0 + b, :]
                            .rearrange("c r w -> c (r w)"),
                            OT[h][:, a:b, :].rearrange("p r w -> p (r w)"),
                        )

    nc.compile()
    return nc


def kernel(x, conv_w, A, D, bias, perm):
    x = np.asarray(x)
    wts, bvec = _fused_weights(conv_w, A, D, bias, perm)
    wts_bf = wts.astype(ml_dtypes.bfloat16)

    # host pre-pad: zero guard row/col on each side, bf16
    xpad = np.zeros((B, CIN, Hp, Wp), ml_dtypes.bfloat16)
    xpad[:, :, 1:H + 1, 1:W + 1] = x.astype(ml_dtypes.bfloat16)

    if "nc" not in _nc_cache:
        _nc_cache["nc"] = _build_program()
    nc = _nc_cache["nc"]

    in_maps = [
        {"xb": xpad[c * BS:(c + 1) * BS], "wts": wts_bf, "bvec": bvec}
        for c in range(NCORES)
    ]
    res = run_bass_kernel_spmd(nc, in_maps, core_ids=list(range(NCORES)))
    _nc_cache["last_result"] = res

    out = np.empty((B, 64, H, W), np.complex64)
    for c in range(NCORES):
        o = np.asarray(res.results[c]["out"])        # (BS,128,H,Wp) bf16
        o = o[:, :, :, 1:W + 1].astype(np.float32)
        out[c * BS:(c + 1) * BS].real = o[:, :64]
        out[c * BS:(c + 1) * BS].imag = o[:, 64:]
    return out


# revision 24
# speedup vs baseline: 1.0232x; 1.0003x over previous
"""ACDC2D fused kernel for 8 Trainium2 NeuronCores.

The reference module is: grouped 3x3 conv (64ch, 8 groups) -> *A ->
FFT(channel) -> *D -> IFFT(channel) -> +bias -> channel permutation ->
/sqrt(64), producing a complex64 output.

Because ifft(D * fft(z)) along a length-64 axis is multiplication by the
circulant matrix C = circ(ifft(D)), the whole module collapses to a single
dense 3x3 convolution with 64 input channels and 128 output channels
(real part || imag part), whose weights are precomputed on the host from
(conv_w, A, D, perm); the permutation, A, D and 1/8 scale all fold into
the weights, and the (real) bias is added during PSUM evacuation.

Device kernel (per core, data-parallel over batch: 2 samples/core):
  - The host pre-pads x to 194x194 (one zero guard row/col each side) in
    bf16, so every conv tap is a pure flat offset into SBUF and every DMA
    is one contiguous multi-KB run per partition.
  - Sample 0 lives on SBUF partitions 0-63, sample 1 on partitions
    64-127. Each conv tap is a K=64 matmul on its own PE row-group
    (tile_position (0,0) / (64,0)); interleaving the two samples' taps
    makes consecutive matmuls run CONCURRENTLY on the two halves of the
    128x128 array, so 18 matmuls take ~9 passes of the free dim.
    Input is read once, no duplicated copies, no shifted SBUF copies.
  - Each sample's 9 taps accumulate into that sample's own PSUM bank per
    512-col chunk; ScalarE evacuates sample 0 and VectorE sample 1
    (PSUM -> SBUF bf16, + per-channel bias), DMA writes padded rows out.
Host post-processing strips the pad and assembles complex64.
"""

import numpy as np
import ml_dtypes

import concourse.bass as bass
import concourse.bacc as bacc
import concourse.tile as tile
from concourse import mybir
from concourse.bass_utils import run_bass_kernel_spmd

# ---- problem geometry (hardcoded, matches setup_inputs) ----
B, CIN, H, W = 16, 64, 192, 192
COUT = 128               # 64 real + 64 imag output channels
NCORES = 8
BS = B // NCORES         # samples per core (2: one per partition half)
Wp = W + 1               # padded row width (SHARED guard: right guard of row h = left guard of row h+1)
Hp = H + 2               # padded height (guard row top/bottom)
TH = 32                  # output rows per strip
NR = TH + 4              # row slots per SBUF region (1 pad front/back)
NSTRIP = H // TH
SPAN0, SPAN1 = 2 * Wp, (TH + 2) * Wp   # matmul span within a strip buffer
CHUNK = 512              # psum bank / matmul free-dim limit (fp32 psum)
ROWS_A = 8               # rows in the first input-DMA piece (startup pipelining)

BF16 = mybir.dt.bfloat16
F32 = mybir.dt.float32

TAPS = [(dh, dw) for dh in (-1, 0, 1) for dw in (-1, 0, 1)]

_nc_cache = {}


def _fused_weights(conv_w, A, D, bias, perm):
    """Fold conv + A + FFT*D*IFFT + perm + 1/8 into dense conv weights.

    Returns (wts[k=128, tap=9, o=128] f32 — identical halves, and
    bvec[128,1] f32).
    """
    A = np.asarray(A, np.float64).reshape(CIN)
    D = np.asarray(D, np.float64).reshape(CIN)
    bias = np.asarray(bias, np.float64).reshape(CIN)
    perm = np.asarray(perm).reshape(CIN)
    conv_w = np.asarray(conv_w, np.float64)          # (64, 8, 3, 3)

    c = np.fft.ifft(D)
    idx = (np.arange(CIN)[:, None] - np.arange(CIN)[None, :]) % CIN
    M = (c[idx] * A[None, :])[perm, :] / 8.0          # complex (64,64)
    bias_p = (bias[perm] / 8.0).astype(np.float32)

    W_d = np.zeros((CIN, CIN, 3, 3))
    for co in range(CIN):
        g = co // 8
        W_d[co, g * 8:(g + 1) * 8] = conv_w[co]
    U = np.einsum("oc,cikl->oikl", M, W_d)            # complex (64,64,3,3)
    W2 = np.concatenate([np.real(U), np.imag(U)], axis=0).astype(np.float32)

    wts = np.zeros((128, 9, 128), np.float32)         # [k, tap, o]
    for t, (dh, dw) in enumerate(TAPS):
        lt = W2[:, :, dh + 1, dw + 1].T               # [i, o]
        wts[:64, t, :] = lt
        wts[64:, t, :] = lt
    bvec = np.concatenate([bias_p, np.zeros(64, np.float32)]).reshape(128, 1)
    return wts, bvec


def _build_program():
    nc = bacc.Bacc("TRN2", target_bir_lowering=False, debug=False)

    x_d = nc.declare_dram_parameter("xb", [BS, CIN, Hp, Wp], BF16, False)
    w_d = nc.declare_dram_parameter("wts", [128, 9, 128], BF16, False)
    b_d = nc.declare_dram_parameter("bvec", [128, 1], F32, False)
    o_d = nc.declare_dram_parameter("out", [BS, COUT, H, Wp], BF16, True)

    with tile.TileContext(nc) as tc:
        with (
            tc.tile_pool(name="const", bufs=1) as constp,
            tc.tile_pool(name="rin", bufs=3) as rinp,
            tc.tile_pool(name="outp", bufs=4) as outp,
            tc.tile_pool(name="psum", bufs=4, space=bass.MemorySpace.PSUM) as psp,
        ):
            # weights first on the fast sync queue (first matmul needs them);
            # bias on gpsimd (only the first evacuation needs it)
            w_sb = constp.tile([128, 9, 128], BF16)
            nc.sync.dma_start(w_sb[:], w_d[:])
            b_sb = constp.tile([128, 1], F32)
            nc.gpsimd.dma_start(b_sb[:], b_d[:])

            # PE warmup: dep-free matmuls on scratch garbage release the
            # HAM clock gate (1.2 -> 2.4 GHz) while the first input DMAs
            # are still in flight, so real matmuls start warm.
            warm_w = constp.tile([128, 128], BF16)
            nc.gpsimd.memset(warm_w[:], 0.0)
            warm_x = constp.tile([128, CHUNK], BF16)
            nc.gpsimd.memset(warm_x[:], 0.0)
            for wi in range(8):
                warm_ps = psp.tile([128, CHUNK], F32, tag=f"ps{wi % 2}",
                                   name="warmps")
                nc.tensor.matmul(warm_ps[:], warm_w[:], warm_x[:],
                                 start=True, stop=True)

            for si in range(NSTRIP):
                r0 = si * TH

                R = rinp.tile([128, NR, Wp], BF16, tag="rin")
                # slots 1..TH+2 <- padded rows r0..r0+TH+1, one sample per
                # partition half — (s c) merges contiguously into the
                # partition dim, so one DMA covers both samples. Two row
                # pieces so the first chunks' matmuls start before the
                # whole strip has landed.
                nc.sync.dma_start(
                    R[:, 1:1 + ROWS_A, :].rearrange("p r w -> p (r w)"),
                    x_d[:, :, r0:r0 + ROWS_A, :]
                    .rearrange("s c r w -> (s c) (r w)"),
                )
                nc.sync.dma_start(
                    R[:, 1 + ROWS_A:TH + 3, :].rearrange("p r w -> p (r w)"),
                    x_d[:, :, r0 + ROWS_A:r0 + TH + 2, :]
                    .rearrange("s c r w -> (s c) (r w)"),
                )
                # the one guard cell past the last loaded row (read by the
                # (+1,+1) tap at wo=191 of the last output row) must be zero
                nc.gpsimd.memset(R[:, TH + 3:TH + 4, 0:1], 0.0)
                Rf = R[:].rearrange("p r c -> p (r c)")

                # one staging tile for BOTH samples: each output piece is
                # then a single DMA (dram AP permuted to (c s r w)), halving
                # the ~0.6us-per-DMA dispatch cost on the tail path
                OT = outp.tile([128, BS, TH, Wp], BF16, tag="out", name="ot")
                OTf = [OT[:, h, :, :].rearrange("p r c -> p (r c)")
                       for h in range(BS)]

                pos = SPAN0
                while pos < SPAN1:
                    rem = SPAN1 - pos
                    # avoid a tiny ragged tail chunk (instruction-floor
                    # bound): split the last 512+32 into two 272s
                    n = 272 if rem == CHUNK + 32 else min(CHUNK, rem)
                    ps = [psp.tile([128, CHUNK], F32, tag=f"ps{h}", name=f"ps{h}")
                          for h in range(BS)]
                    for t, (dh, dw) in enumerate(TAPS):
                        off = dh * Wp + dw
                        for h in range(BS):
                            p0 = 64 * h
                            nc.tensor.matmul(
                                ps[h][:, 0:n],
                                w_sb[p0:p0 + 64, t, :],
                                Rf[p0:p0 + 64, pos + off:pos + off + n],
                                start=(t == 0),
                                stop=(t == 8),
                            )
                    lo = pos - SPAN0
                    nc.scalar.activation(
                        OTf[0][:, lo:lo + n], ps[0][:, 0:n],
                        mybir.ActivationFunctionType.Identity,
                        bias=b_sb[:, 0:1],
                    )
                    nc.vector.tensor_scalar_add(
                        OTf[1][:, lo:lo + n], ps[1][:, 0:n], b_sb[:, 0:1]
                    )
                    pos += n

                # output in row-pieces; range-precise dep tracking releases
                # each piece as soon as its chunks are evacuated. Finer
                # pieces on the last strip shrink the post-compute tail.
                cuts = [0, 16, 28, TH] if si == NSTRIP - 1 else [0, 16, TH]
                for a, b in zip(cuts, cuts[1:]):
                    nc.sync.dma_start(
                        o_d[:, :, r0 + a:r0 + b, :]
                        .rearrange("s c r w -> c s r w"),
                        OT[:, :, a:b, :],
                    )

    nc.compile()
    return nc


def kernel(x, conv_w, A, D, bias, perm):
    x = np.asarray(x)
    wts, bvec = _fused_weights(conv_w, A, D, bias, perm)
    wts_bf = wts.astype(ml_dtypes.bfloat16)

    # host pre-pad: zero guard row/col on each side, bf16
    xpad = np.zeros((B, CIN, Hp, Wp), ml_dtypes.bfloat16)
    xpad[:, :, 1:H + 1, 1:W + 1] = x.astype(ml_dtypes.bfloat16)

    if "nc" not in _nc_cache:
        _nc_cache["nc"] = _build_program()
    nc = _nc_cache["nc"]

    in_maps = [
        {"xb": xpad[c * BS:(c + 1) * BS], "wts": wts_bf, "bvec": bvec}
        for c in range(NCORES)
    ]
    res = run_bass_kernel_spmd(nc, in_maps, core_ids=list(range(NCORES)))
    _nc_cache["last_result"] = res

    out = np.empty((B, 64, H, W), np.complex64)
    for c in range(NCORES):
        o = np.asarray(res.results[c]["out"])        # (BS,128,H,Wp) bf16
        o = o[:, :, :, 1:W + 1].astype(np.float32)
        out[c * BS:(c + 1) * BS].real = o[:, :64]
        out[c * BS:(c + 1) * BS].imag = o[:, 64:]
    return out
